# revision 10
# baseline (speedup 1.0000x reference)
# Trainium2 Bass kernel for nn_BlockRecurrentModel (block-recurrent GRU cell).
#
# Sharding: pure data-parallel over the flattened (B*T)=4096 batch rows —
# 512 rows per core on 8 cores, weights replicated, zero collectives.
#
# Per-core layout: activations live rows-on-partition ([128 rows, feats]).
# Every matmul uses a PE-transposed activation tile as the stationary
# operand (lhsT, [128 feats, 128 rows]) and streams weight tiles as the
# moving operand ([128, 512]).  All matmuls run in bf16 (1 cycle/row on
# TRN2 PE) with fp32 PSUM accumulation; LayerNorm stats run in fp32 via
# bn_stats/bn_aggr, and the LN apply is fused into the SiLU activation as
# silu(x * rstd + (-mean*rstd)) using per-partition scale/bias.
#
# NOTE: the reference's LN gains/biases (ln_*_g, ln_*_b) and gate bias
# b_gate are constants ones/zeros from setup_inputs(); the device kernel
# folds them out (multiply-by-1 / add-0 are exact no-ops).  kernel()
# verifies this at runtime and fails loudly if it ever changes.

import numpy as np
import ml_dtypes

import concourse.bass as bass
import concourse.bacc as bacc
import concourse.mybir as mybir
import concourse.tile as tile
from concourse.bass_utils import run_bass_kernel_spmd
from concourse.masks import make_identity

# Problem dims (hardcoded from the problem spec).
STOCH, ACTD, HID, DETER, G = 1024, 256, 1024, 4096, 8
BH = DETER // G              # 512
BLK_IN = 3 * HID + BH        # 3584
B, T = 64, 64
N_ROWS = B * T               # 4096 flattened rows
NCORES = 8
R = N_ROWS // NCORES         # 512 rows per core
P = 128
NRT = R // P                 # 4 row-tiles per core
EPS = 1e-3

F32 = mybir.dt.float32
BF16 = mybir.dt.bfloat16
AF = mybir.ActivationFunctionType
ALU = mybir.AluOpType
NP_BF16 = ml_dtypes.bfloat16


def _emit(nc, tc, io):
    s_bf, a_bf, d_bfh = io["s_bf"], io["a_bf"], io["d_bf"]
    d_f32 = io["d_f32"]
    Ws, Wa, Wd, Whid, Wgate = io["Ws"], io["Wa"], io["Wd"], io["Whid"], io["Wgate"]
    out = io["out"]

    from contextlib import ExitStack

    with ExitStack() as ctx:
        singles = ctx.enter_context(tc.tile_pool(name="singles", bufs=1))
        identity = singles.tile([P, P], BF16)
        make_identity(nc, identity)
        eps_t = singles.tile([P, 1], F32)
        nc.vector.memset(eps_t, EPS)
        neg1_t = singles.tile([P, 1], F32)
        nc.vector.memset(neg1_t, -1.0)
        zero_t = singles.tile([P, 1], F32)
        nc.vector.memset(zero_t, 0.0)

        psum_mm = ctx.enter_context(tc.tile_pool(name="psum_mm", bufs=4, space="PSUM"))
        psum_tp = ctx.enter_context(tc.tile_pool(name="psum_tp", bufs=3, space="PSUM"))
        stats_pool = ctx.enter_context(tc.tile_pool(name="stats", bufs=8))
        wpool = ctx.enter_context(tc.tile_pool(name="wpool", bufs=3))

        # Persistent transposed activations (bf16, [feat-part, ktile, row]).
        dT_pool = ctx.enter_context(tc.tile_pool(name="dT_pool", bufs=NRT))
        xT_pool = ctx.enter_context(tc.tile_pool(name="xT_pool", bufs=NRT))
        dT = [dT_pool.tile([P, DETER // P, P], BF16, name=f"dT{rt}", tag="dT")
              for rt in range(NRT)]
        xT = [xT_pool.tile([P, 3 * HID // P, P], BF16, name=f"xT{rt}", tag="xT")
              for rt in range(NRT)]

        def ln_scale_bias(stats_ap, tag_sfx):
            """stats [P, n, 6] -> (rstd [P,1], -mean*rstd [P,1]) fp32 tiles."""
            mv = stats_pool.tile([P, 2], F32, name=f"mv_{tag_sfx}", tag="mv")
            nc.vector.bn_aggr(out=mv, in_=stats_ap)
            std = stats_pool.tile([P, 1], F32, name=f"std_{tag_sfx}", tag="std")
            nc.scalar.activation(out=std, in_=mv[:, 1:2], func=AF.Sqrt,
                                 bias=eps_t, scale=1.0)
            rstd = stats_pool.tile([P, 1], F32, name=f"rstd_{tag_sfx}", tag="rstd")
            nc.vector.reciprocal(rstd, std)
            nmr = stats_pool.tile([P, 1], F32, name=f"nmr_{tag_sfx}", tag="nmr")
            nc.vector.tensor_scalar(out=nmr, in0=mv[:, 0:1], scalar1=rstd,
                                    scalar2=-1.0, op0=ALU.mult, op1=ALU.mult)
            return rstd, nmr

        def transpose_into(dst_ap, src_ap, drain_engine):
            """PE-transpose src [P, P] bf16 -> dst [P, P] via PSUM."""
            ps = psum_tp.tile([P, P], BF16, name="tp_ps", tag="tp")
            nc.tensor.transpose(ps, src_ap, identity)
            if drain_engine is nc.scalar:
                nc.scalar.copy(dst_ap, ps)
            else:
                drain_engine.tensor_copy(dst_ap, ps)

        # ---------------- Phase A: input staging + dense branches ----------------
        with ExitStack() as actx:
            ain = ctx  # persistent parent
            in_pool = actx.enter_context(tc.tile_pool(name="in_pool", bufs=NRT))
            sT_pool = actx.enter_context(tc.tile_pool(name="sT_pool", bufs=NRT))
            x_raw_pool = actx.enter_context(tc.tile_pool(name="x_raw_pool", bufs=6))
            acts_pool = actx.enter_context(tc.tile_pool(name="acts_pool", bufs=NRT))

            sT = [sT_pool.tile([P, STOCH // P, P], BF16, name=f"sT{rt}", tag="sT")
                  for rt in range(NRT)]
            aT = [sT_pool.tile([P, ACTD // P, P], BF16, name=f"aT{rt}", tag="aT")
                  for rt in range(NRT)]
            acts_rm = [acts_pool.tile([P, 3 * HID], BF16, name=f"acts{rt}", tag="acts")
                       for rt in range(NRT)]

            # Stage + transpose all inputs.
            for rt in range(NRT):
                rs = slice(rt * P, (rt + 1) * P)
                s_st = in_pool.tile([P, STOCH], BF16, name="s_st", tag="s_st")
                nc.sync.dma_start(out=s_st, in_=s_bf[rs, :])
                a_st = in_pool.tile([P, ACTD], BF16, name="a_st", tag="a_st")
                nc.sync.dma_start(out=a_st, in_=a_bf[rs, :])
                d_st = in_pool.tile([P, DETER], BF16, name="d_st", tag="d_st")
                nc.sync.dma_start(out=d_st, in_=d_bfh[rs, :])
                for k in range(STOCH // P):
                    transpose_into(sT[rt][:, k, :], s_st[:, k * P:(k + 1) * P],
                                   nc.vector)
                for k in range(ACTD // P):
                    transpose_into(aT[rt][:, k, :], a_st[:, k * P:(k + 1) * P],
                                   nc.vector)
                for k in range(DETER // P):
                    transpose_into(dT[rt][:, k, :], d_st[:, k * P:(k + 1) * P],
                                   nc.vector)

            # Dense branches: out column ranges in acts_rm are [s | a | d].
            branches = [
                ("s", sT, STOCH // P, Ws, 0),
                ("a", aT, ACTD // P, Wa, HID),
                ("d", dT, DETER // P, Wd, 2 * HID),
            ]
            MCH = HID // 512  # 2 m-chunks of 512
            for bname, lT, KT, W, coff in branches:
                stats = [stats_pool.tile([P, MCH, 6], F32, name=f"stA_{bname}{rt}",
                                         tag="statsA") for rt in range(NRT)]
                x_raw = [x_raw_pool.tile([P, HID], BF16, name=f"xr_{bname}{rt}",
                                         tag="x_raw") for rt in range(NRT)]
                for mc in range(MCH):
                    # Weight chunk [P, kn, 512], one DMA per chunk.  One tag
                    # per chunk shape and always fully written: keeps slot
                    # history uniform so Tile emits <=2 sync waits per DMA
                    # (walrus DIRECT2D descriptor limit).
                    KCH = 8  # k-tiles per weight DMA chunk
                    pas = [None] * NRT
                    for k0 in range(0, KT, KCH):
                        kn = min(KCH, KT - k0)
                        wt = wpool.tile([P, kn, 512], BF16, name=f"w_{bname}",
                                        tag=f"w{kn}")
                        src = W[k0 * P:(k0 + kn) * P, mc * 512:(mc + 1) * 512]
                        nc.gpsimd.dma_start(
                            out=wt[:, :kn, :],
                            in_=src.rearrange("(kk p) m -> p kk m", p=P))
                        for k in range(kn):
                            for rt in range(NRT):
                                if k0 + k == 0:
                                    pas[rt] = psum_mm.tile([P, 512], F32,
                                                           name="paA", tag="mm")
                                nc.tensor.matmul(pas[rt],
                                                 lhsT=lT[rt][:, k0 + k, :],
                                                 rhs=wt[:, k, :],
                                                 start=(k0 + k == 0),
                                                 stop=(k0 + k == KT - 1))
                    for rt in range(NRT):
                        nc.vector.bn_stats(out=stats[rt][:, mc, :], in_=pas[rt])
                        nc.scalar.copy(out=x_raw[rt][:, mc * 512:(mc + 1) * 512],
                                       in_=pas[rt])
                for rt in range(NRT):
                    rstd, nmr = ln_scale_bias(stats[rt], f"A{bname}{rt}")
                    nc.scalar.activation(out=acts_rm[rt][:, coff:coff + HID],
                                         in_=x_raw[rt], func=AF.Silu,
                                         bias=nmr, scale=rstd)

            # Transpose fused activations for the block matmul.
            for rt in range(NRT):
                for k in range(3 * HID // P):
                    transpose_into(xT[rt][:, k, :], acts_rm[rt][:, k * P:(k + 1) * P],
                                   nc.vector)

        # ---------------- Phase B: block-diagonal hidden matmul + LN + silu -------
        hT_pool = ctx.enter_context(tc.tile_pool(name="hT_pool", bufs=NRT))
        hT = [hT_pool.tile([P, DETER // P, P], BF16, name=f"hT{rt}", tag="hT")
              for rt in range(NRT)]
        with ExitStack() as bctx:
            h_raw_pool = bctx.enter_context(tc.tile_pool(name="h_raw_pool", bufs=NRT))
            h_ln_pool = bctx.enter_context(tc.tile_pool(name="h_ln_pool", bufs=2))
            h_raw = [h_raw_pool.tile([P, DETER], BF16, name=f"hraw{rt}", tag="h_raw")
                     for rt in range(NRT)]
            statsB = [stats_pool.tile([P, G, 6], F32, name=f"stB{rt}", tag="statsB")
                      for rt in range(NRT)]
            KB = BLK_IN // P  # 28 k-tiles per block: 4 from d_g, 24 shared
            for g in range(G):
                pbs = [None] * NRT
                KCH = 4
                for k0 in range(0, KB, KCH):
                    kn = min(KCH, KB - k0)
                    wt = wpool.tile([P, kn, 512], BF16, name="w_hid",
                                    tag=f"w{kn}", bufs=4)
                    src = Whid[g, k0 * P:(k0 + kn) * P, :]
                    nc.gpsimd.dma_start(out=wt[:, :kn, :],
                                        in_=src.rearrange("(kk p) m -> p kk m", p=P))
                    for k in range(kn):
                        kk = k0 + k
                        for rt in range(NRT):
                            if kk == 0:
                                pbs[rt] = psum_mm.tile([P, 512], F32,
                                                       name="paB", tag="mm")
                            lhsT = (dT[rt][:, g * 4 + kk, :] if kk < 4
                                    else xT[rt][:, kk - 4, :])
                            nc.tensor.matmul(pbs[rt], lhsT=lhsT, rhs=wt[:, k, :],
                                             start=(kk == 0), stop=(kk == KB - 1))
                for rt in range(NRT):
                    nc.vector.bn_stats(out=statsB[rt][:, g, :], in_=pbs[rt])
                    nc.scalar.copy(out=h_raw[rt][:, g * 512:(g + 1) * 512],
                                   in_=pbs[rt])
            for rt in range(NRT):
                rstd, nmr = ln_scale_bias(statsB[rt], f"B{rt}")
                h_ln = h_ln_pool.tile([P, DETER], BF16, name="h_ln", tag="h_ln")
                nc.scalar.activation(out=h_ln, in_=h_raw[rt], func=AF.Silu,
                                     bias=nmr, scale=rstd)
                for k in range(DETER // P):
                    transpose_into(hT[rt][:, k, :], h_ln[:, k * P:(k + 1) * P],
                                   nc.scalar)

        # ---------------- Phase C: gates + GRU blend ----------------
        with ExitStack() as cctx:
            gate_pool = cctx.enter_context(tc.tile_pool(name="gate_pool", bufs=6))
            blend_pool = cctx.enter_context(tc.tile_pool(name="blend_pool", bufs=4))
            dre_pool = cctx.enter_context(tc.tile_pool(name="dre_pool", bufs=4))
            KC = BH // P  # 4 k-tiles
            for g in range(G):
                r_sb = [None] * NRT
                u_sb = [None] * NRT
                c_sb = [None] * NRT
                for mc, gate in enumerate(("r", "u", "c")):
                    wt = wpool.tile([P, KC, 512], BF16, name="w_gate",
                                    tag=f"w{KC}", bufs=4)
                    src = Wgate[g, :, mc * 512:(mc + 1) * 512]
                    nc.gpsimd.dma_start(out=wt[:, :KC, :],
                                        in_=src.rearrange("(kk p) m -> p kk m", p=P))
                    pcs = [None] * NRT
                    for k in range(KC):
                        for rt in range(NRT):
                            if k == 0:
                                pcs[rt] = psum_mm.tile([P, 512], F32,
                                                       name="paC", tag="mm")
                            nc.tensor.matmul(pcs[rt], lhsT=hT[rt][:, g * 4 + k, :],
                                             rhs=wt[:, k, :],
                                             start=(k == 0), stop=(k == KC - 1))
                    for rt in range(NRT):
                        if gate == "r":
                            r_sb[rt] = gate_pool.tile([P, 512], F32,
                                                      name="r_sb", tag="r")
                            nc.scalar.activation(out=r_sb[rt], in_=pcs[rt],
                                                 func=AF.Sigmoid, bias=zero_t)
                        elif gate == "u":
                            u_sb[rt] = gate_pool.tile([P, 512], F32,
                                                      name="u_sb", tag="u")
                            nc.scalar.activation(out=u_sb[rt], in_=pcs[rt],
                                                 func=AF.Sigmoid, bias=neg1_t)
                        else:
                            rc = blend_pool.tile([P, 512], F32, name="rc", tag="rc")
                            nc.vector.tensor_mul(rc, r_sb[rt], pcs[rt])
                            c_sb[rt] = gate_pool.tile([P, 512], F32,
                                                      name="c_sb", tag="c")
                            nc.scalar.activation(out=c_sb[rt], in_=rc, func=AF.Tanh,
                                                 bias=zero_t)
                # blend: d_new = d + u*(c - d)
                for rt in range(NRT):
                    rs = slice(rt * P, (rt + 1) * P)
                    cs = slice(g * 512, (g + 1) * 512)
                    dre = dre_pool.tile([P, 512], F32, name="dre", tag="dre")
                    nc.gpsimd.dma_start(out=dre, in_=d_f32[rs, cs])
                    t = blend_pool.tile([P, 512], F32, name="t_blend", tag="t")
                    nc.vector.tensor_sub(t, c_sb[rt], dre)
                    nc.vector.tensor_mul(t, u_sb[rt], t)
                    nc.vector.tensor_add(t, t, dre)
                    nc.gpsimd.dma_start(out=out[rs, cs], in_=t)


def build_nc():
    nc = bacc.Bacc()
    io = {
        "s_bf": nc.declare_dram_parameter("s_bf", [R, STOCH], BF16, isOutput=False),
        "a_bf": nc.declare_dram_parameter("a_bf", [R, ACTD], BF16, isOutput=False),
        "d_bf": nc.declare_dram_parameter("d_bf", [R, DETER], BF16, isOutput=False),
        "d_f32": nc.declare_dram_parameter("d_f32", [R, DETER], F32, isOutput=False),
        "Ws": nc.declare_dram_parameter("Ws", [STOCH, HID], BF16, isOutput=False),
        "Wa": nc.declare_dram_parameter("Wa", [ACTD, HID], BF16, isOutput=False),
        "Wd": nc.declare_dram_parameter("Wd", [DETER, HID], BF16, isOutput=False),
        "Whid": nc.declare_dram_parameter("Whid", [G, BLK_IN, BH], BF16,
                                          isOutput=False),
        "Wgate": nc.declare_dram_parameter("Wgate", [G, BH, 3 * BH], BF16,
                                           isOutput=False),
        "out": nc.declare_dram_parameter("out", [R, DETER], F32, isOutput=True),
    }
    aps = {k: v[:] for k, v in io.items()}
    with tile.TileContext(nc) as tc:
        _emit(nc, tc, aps)
    nc.compile()
    return nc


_NC = None


def _get_nc():
    global _NC
    if _NC is None:
        _NC = build_nc()
    return _NC


def make_in_maps(inputs):
    s = np.ascontiguousarray(np.asarray(inputs["s"], np.float32).reshape(N_ROWS, STOCH))
    a = np.ascontiguousarray(np.asarray(inputs["a"], np.float32).reshape(N_ROWS, ACTD))
    d = np.ascontiguousarray(np.asarray(inputs["d"], np.float32).reshape(N_ROWS, DETER))

    # The device kernel folds out LN gains/biases and the gate bias, which are
    # constants (ones/zeros) in this problem.  Verify.
    for nm, want in [("ln_s_g", 1), ("ln_a_g", 1), ("ln_d_g", 1), ("ln_h_g", 1),
                     ("ln_s_b", 0), ("ln_a_b", 0), ("ln_d_b", 0), ("ln_h_b", 0),
                     ("b_gate", 0)]:
        v = np.asarray(inputs[nm], np.float32)
        if not np.all(v == want):
            raise ValueError(f"kernel assumes {nm} == {want}; got varying values")

    w_bf = {
        "Ws": np.asarray(inputs["W_s"], np.float32).astype(NP_BF16),
        "Wa": np.asarray(inputs["W_a"], np.float32).astype(NP_BF16),
        "Wd": np.asarray(inputs["W_d"], np.float32).astype(NP_BF16),
        "Whid": np.asarray(inputs["W_hid"], np.float32).astype(NP_BF16),
        "Wgate": np.asarray(inputs["W_gate"], np.float32).astype(NP_BF16),
    }
    in_maps = []
    for c in range(NCORES):
        rows = slice(c * R, (c + 1) * R)
        in_maps.append({
            "s_bf": s[rows].astype(NP_BF16),
            "a_bf": a[rows].astype(NP_BF16),
            "d_bf": d[rows].astype(NP_BF16),
            "d_f32": d[rows],
            **w_bf,
        })
    return in_maps


def run(inputs, **spmd_kwargs):
    nc = _get_nc()
    in_maps = make_in_maps(inputs)
    res = run_bass_kernel_spmd(nc, in_maps, core_ids=list(range(NCORES)),
                               **spmd_kwargs)
    outs = [np.asarray(res.results[c]["out"], np.float32) for c in range(NCORES)]
    full = np.concatenate(outs, axis=0).reshape(B, T, DETER)
    return full, res


def kernel(**inputs) -> np.ndarray:
    full, _ = run(inputs)
    return full


# revision 18
# speedup vs baseline: 21.3695x; 21.3695x over previous
# Trainium2 Bass kernel for nn_BlockRecurrentModel (block-recurrent GRU cell).
#
# Sharding: pure data-parallel over the flattened (B*T)=4096 batch rows —
# 512 rows per core on 8 cores, weights replicated, zero collectives.
#
# Per-core layout: activations live rows-on-partition ([128 rows, feats]).
# Every matmul uses a PE-transposed activation tile as the stationary
# operand (lhsT, [128 feats, 128 rows]) and streams weight tiles as the
# moving operand ([128, 512]).  All matmuls run in bf16 (1 cycle/row on
# TRN2 PE) with fp32 PSUM accumulation; LayerNorm stats run in fp32 via
# bn_stats/bn_aggr, and the LN apply is fused into the SiLU activation as
# silu(x * rstd + (-mean*rstd)) using per-partition scale/bias.
#
# NOTE: the reference's LN gains/biases (ln_*_g, ln_*_b) and gate bias
# b_gate are constants ones/zeros from setup_inputs(); the device kernel
# folds them out (multiply-by-1 / add-0 are exact no-ops).  kernel()
# verifies this at runtime and fails loudly if it ever changes.

from contextlib import ExitStack

import numpy as np
import ml_dtypes

import concourse.bass as bass
import concourse.bacc as bacc
import concourse.mybir as mybir
import concourse.tile as tile
from concourse.bass_utils import run_bass_kernel_spmd
from concourse.masks import make_identity

# Problem dims (hardcoded from the problem spec).
STOCH, ACTD, HID, DETER, G = 1024, 256, 1024, 4096, 8
BH = DETER // G              # 512
BLK_IN = 3 * HID + BH        # 3584
B, T = 64, 64
N_ROWS = B * T               # 4096 flattened rows
NCORES = 8
R = N_ROWS // NCORES         # 512 rows per core
P = 128
NRT = R // P                 # 4 row-tiles per core
EPS = 1e-3

F32 = mybir.dt.float32
BF16 = mybir.dt.bfloat16
AF = mybir.ActivationFunctionType
ALU = mybir.AluOpType
NP_BF16 = ml_dtypes.bfloat16


def _emit(nc, tc, io):
    s_bf, a_bf, d_bfh = io["s_bf"], io["a_bf"], io["d_bf"]
    d_f32 = io["d_f32"]
    Ws, Wa, Wd, Whid, Wgate = io["Ws"], io["Wa"], io["Wd"], io["Whid"], io["Wgate"]
    out = io["out"]

    with ExitStack() as ctx:
        singles = ctx.enter_context(tc.tile_pool(name="singles", bufs=1))
        identity = singles.tile([P, P], BF16)
        make_identity(nc, identity)
        eps_t = singles.tile([P, 1], F32)
        nc.vector.memset(eps_t, EPS)
        neg1_t = singles.tile([P, 1], F32)
        nc.vector.memset(neg1_t, -1.0)
        zero_t = singles.tile([P, 1], F32)
        nc.vector.memset(zero_t, 0.0)

        psum_mm = ctx.enter_context(tc.tile_pool(name="psum_mm", bufs=5, space="PSUM"))
        stats_pool = ctx.enter_context(tc.tile_pool(name="stats", bufs=8))
        wpool = ctx.enter_context(tc.tile_pool(name="wpool", bufs=3))

        # hT allocated before the AB-scoped pools so dT/xT can be released
        # (LIFO) after phase B while hT persists into phase C.
        hT_pool = ctx.enter_context(tc.tile_pool(name="hT_pool", bufs=NRT))
        hT = [hT_pool.tile([P, DETER // P, P], BF16, name=f"hT{rt}", tag="hT")
              for rt in range(NRT)]

        def ln_scale_bias(stats_ap, tag_sfx):
            """stats [P, n, 6] -> (rstd [P,1], -mean*rstd [P,1]) fp32 tiles."""
            mv = stats_pool.tile([P, 2], F32, name=f"mv_{tag_sfx}", tag="mv")
            nc.vector.bn_aggr(out=mv, in_=stats_ap)
            std = stats_pool.tile([P, 1], F32, name=f"std_{tag_sfx}", tag="std")
            nc.scalar.activation(out=std, in_=mv[:, 1:2], func=AF.Sqrt,
                                 bias=eps_t, scale=1.0)
            rstd = stats_pool.tile([P, 1], F32, name=f"rstd_{tag_sfx}", tag="rstd")
            nc.vector.reciprocal(rstd, std)
            nmr = stats_pool.tile([P, 1], F32, name=f"nmr_{tag_sfx}", tag="nmr")
            nc.vector.tensor_scalar(out=nmr, in0=mv[:, 0:1], scalar1=rstd,
                                    scalar2=-1.0, op0=ALU.mult, op1=ALU.mult)
            return rstd, nmr

        def transpose_into(dst_ap, src_ap, drain_engine):
            """PE-transpose src [P, P] bf16 -> dst [P, P] via PSUM."""
            ps = psum_tp.tile([P, P], BF16, name="tp_ps", tag="tp")
            nc.tensor.transpose(ps, src_ap, identity)
            if drain_engine is nc.scalar:
                nc.scalar.copy(dst_ap, ps)
            else:
                drain_engine.tensor_copy(dst_ap, ps)

        with ExitStack() as abctx:
            # Transposed d input and fused-activation tiles live through A+B.
            # The transpose PSUM pool also lives only through A+B so phase C
            # can use its banks for a second matmul pool.
            psum_tp = abctx.enter_context(
                tc.tile_pool(name="psum_tp", bufs=3, space="PSUM"))
            dT_pool = abctx.enter_context(tc.tile_pool(name="dT_pool", bufs=NRT))
            xT_pool = abctx.enter_context(tc.tile_pool(name="xT_pool", bufs=NRT))
            dT = [dT_pool.tile([P, DETER // P, P], BF16, name=f"dT{rt}", tag="dT")
                  for rt in range(NRT)]
            xT = [xT_pool.tile([P, 3 * HID // P, P], BF16, name=f"xT{rt}", tag="xT")
                  for rt in range(NRT)]

            # ---------------- Phase A: input staging + dense branches --------
            with ExitStack() as actx:
                in_pool = actx.enter_context(tc.tile_pool(name="in_pool", bufs=2))
                sT_pool = actx.enter_context(tc.tile_pool(name="sT_pool", bufs=NRT))
                x_raw_pool = actx.enter_context(
                    tc.tile_pool(name="x_raw_pool", bufs=4))
                acts_pool = actx.enter_context(
                    tc.tile_pool(name="acts_pool", bufs=4))

                sT = [sT_pool.tile([P, STOCH // P, P], BF16, name=f"sT{rt}",
                                   tag="sT") for rt in range(NRT)]
                aT = [sT_pool.tile([P, ACTD // P, P], BF16, name=f"aT{rt}",
                                   tag="aT") for rt in range(NRT)]

                # Stage + transpose all inputs.
                for rt in range(NRT):
                    rs = slice(rt * P, (rt + 1) * P)
                    s_st = in_pool.tile([P, STOCH], BF16, name="s_st", tag="s_st")
                    nc.sync.dma_start(out=s_st, in_=s_bf[rs, :])
                    a_st = in_pool.tile([P, ACTD], BF16, name="a_st", tag="a_st")
                    nc.sync.dma_start(out=a_st, in_=a_bf[rs, :])
                    d_st = in_pool.tile([P, DETER], BF16, name="d_st", tag="d_st")
                    nc.sync.dma_start(out=d_st, in_=d_bfh[rs, :])
                    for k in range(STOCH // P):
                        transpose_into(sT[rt][:, k, :], s_st[:, k * P:(k + 1) * P],
                                       nc.vector)
                    for k in range(ACTD // P):
                        transpose_into(aT[rt][:, k, :], a_st[:, k * P:(k + 1) * P],
                                       nc.vector)
                    for k in range(DETER // P):
                        transpose_into(dT[rt][:, k, :], d_st[:, k * P:(k + 1) * P],
                                       nc.vector)

                # Dense branches; xT k-tile ranges are [s | a | d].
                branches = [
                    ("s", sT, STOCH // P, Ws, 0),
                    ("a", aT, ACTD // P, Wa, HID),
                    ("d", dT, DETER // P, Wd, 2 * HID),
                ]
                MCH = HID // 512  # 2 m-chunks of 512
                for bname, lT, KT, W, coff in branches:
                    stats = [stats_pool.tile([P, MCH, 6], F32,
                                             name=f"stA_{bname}{rt}", tag="statsA")
                             for rt in range(NRT)]
                    x_raw = [x_raw_pool.tile([P, HID], BF16,
                                             name=f"xr_{bname}{rt}", tag="x_raw")
                             for rt in range(NRT)]
                    for mc in range(MCH):
                        # Weight chunk [P, kn, 512], one DMA per chunk; one tag
                        # per chunk shape, always fully written (wait budget).
                        KCH = 8
                        pas = [None] * NRT
                        for k0 in range(0, KT, KCH):
                            kn = min(KCH, KT - k0)
                            wt = wpool.tile([P, kn, 512], BF16, name=f"w_{bname}",
                                            tag=f"w{kn}")
                            src = W[k0 * P:(k0 + kn) * P, mc * 512:(mc + 1) * 512]
                            nc.sync.dma_start(
                                out=wt[:, :kn, :],
                                in_=src.rearrange("(kk p) m -> p kk m", p=P))
                            for k in range(kn):
                                for rt in range(NRT):
                                    if k0 + k == 0:
                                        pas[rt] = psum_mm.tile([P, 512], F32,
                                                               name="paA", tag="mm")
                                    nc.tensor.matmul(pas[rt],
                                                     lhsT=lT[rt][:, k0 + k, :],
                                                     rhs=wt[:, k, :],
                                                     start=(k0 + k == 0),
                                                     stop=(k0 + k == KT - 1))
                        for rt in range(NRT):
                            nc.vector.bn_stats(out=stats[rt][:, mc, :], in_=pas[rt])
                            nc.scalar.copy(out=x_raw[rt][:, mc * 512:(mc + 1) * 512],
                                           in_=pas[rt])
                    for rt in range(NRT):
                        rstd, nmr = ln_scale_bias(stats[rt], f"A{bname}{rt}")
                        # silu+LN apply and transpose per 512-col chunk so the
                        # block matmuls can start before the whole row is done.
                        for mc in range(MCH):
                            ach = acts_pool.tile([P, 512], BF16, name="ach",
                                                 tag="ach")
                            nc.scalar.activation(
                                out=ach, in_=x_raw[rt][:, mc * 512:(mc + 1) * 512],
                                func=AF.Silu, bias=nmr, scale=rstd)
                            kbase = (coff + mc * 512) // P
                            for j in range(512 // P):
                                transpose_into(xT[rt][:, kbase + j, :],
                                               ach[:, j * P:(j + 1) * P],
                                               nc.vector)

            # ---------------- Phase B: block-diagonal matmul + LN + silu -----
            with ExitStack() as bctx:
                h_raw_pool = bctx.enter_context(
                    tc.tile_pool(name="h_raw_pool", bufs=NRT))
                h_ln_pool = bctx.enter_context(
                    tc.tile_pool(name="h_ln_pool", bufs=4))
                h_raw = [h_raw_pool.tile([P, DETER], BF16, name=f"hraw{rt}",
                                         tag="h_raw") for rt in range(NRT)]
                statsB = [stats_pool.tile([P, G, 6], F32, name=f"stB{rt}",
                                          tag="statsB") for rt in range(NRT)]
                KB = BLK_IN // P  # 28 k-tiles per block: 4 from d_g, 24 shared
                for g in range(G):
                    pbs = [None] * NRT
                    KCH = 4
                    for k0 in range(0, KB, KCH):
                        kn = min(KCH, KB - k0)
                        wt = wpool.tile([P, kn, 512], BF16, name="w_hid",
                                        tag=f"w{kn}", bufs=8)
                        src = Whid[g, k0 * P:(k0 + kn) * P, :]
                        nc.sync.dma_start(
                            out=wt[:, :kn, :],
                            in_=src.rearrange("(kk p) m -> p kk m", p=P))
                        for k in range(kn):
                            kk = k0 + k
                            for rt in range(NRT):
                                if kk == 0:
                                    pbs[rt] = psum_mm.tile([P, 512], F32,
                                                           name="paB", tag="mm")
                                lhsT = (dT[rt][:, g * 4 + kk, :] if kk < 4
                                        else xT[rt][:, kk - 4, :])
                                nc.tensor.matmul(pbs[rt], lhsT=lhsT,
                                                 rhs=wt[:, k, :],
                                                 start=(kk == 0),
                                                 stop=(kk == KB - 1))
                    for rt in range(NRT):
                        nc.vector.bn_stats(out=statsB[rt][:, g, :], in_=pbs[rt])
                        nc.scalar.copy(out=h_raw[rt][:, g * 512:(g + 1) * 512],
                                       in_=pbs[rt])
                for rt in range(NRT):
                    rstd, nmr = ln_scale_bias(statsB[rt], f"B{rt}")
                    for g in range(G):
                        hch = h_ln_pool.tile([P, 512], BF16, name="hch", tag="hch")
                        nc.scalar.activation(
                            out=hch, in_=h_raw[rt][:, g * 512:(g + 1) * 512],
                            func=AF.Silu, bias=nmr, scale=rstd)
                        for j in range(512 // P):
                            transpose_into(hT[rt][:, g * 4 + j, :],
                                           hch[:, j * P:(j + 1) * P], nc.scalar)
        # dT/xT released here.

        # ---------------- Phase C: gates + GRU blend ----------------
        with ExitStack() as cctx:
            psum_c = cctx.enter_context(
                tc.tile_pool(name="psum_c", bufs=3, space="PSUM"))
            gate_pool = cctx.enter_context(tc.tile_pool(name="gate_pool", bufs=8))
            blend_pool = cctx.enter_context(tc.tile_pool(name="blend_pool", bufs=8))
            dre_pool = cctx.enter_context(tc.tile_pool(name="dre_pool", bufs=8))
            KC = BH // P  # 4 k-tiles
            for g in range(G):
                r_sb = [None] * NRT
                u_sb = [None] * NRT
                c_sb = [None] * NRT
                for mc, gate in enumerate(("r", "u", "c")):
                    wt = wpool.tile([P, KC, 512], BF16, name="w_gate",
                                    tag=f"w{KC}", bufs=8)
                    src = Wgate[g, :, mc * 512:(mc + 1) * 512]
                    nc.sync.dma_start(out=wt[:, :KC, :],
                                      in_=src.rearrange("(kk p) m -> p kk m", p=P))
                    pcs = [None] * NRT
                    # alternate psum pools per gate group -> two groups in
                    # flight, PE does not stall on the previous group's drains
                    gpool, gtag = ((psum_mm, "mm") if (g * 3 + mc) % 2 == 0
                                   else (psum_c, "mmc"))
                    for k in range(KC):
                        for rt in range(NRT):
                            if k == 0:
                                pcs[rt] = gpool.tile([P, 512], F32,
                                                     name="paC", tag=gtag)
                            nc.tensor.matmul(pcs[rt], lhsT=hT[rt][:, g * 4 + k, :],
                                             rhs=wt[:, k, :],
                                             start=(k == 0), stop=(k == KC - 1))
                    for rt in range(NRT):
                        if gate == "r":
                            r_sb[rt] = gate_pool.tile([P, 512], F32,
                                                      name="r_sb", tag="r")
                            nc.scalar.activation(out=r_sb[rt], in_=pcs[rt],
                                                 func=AF.Sigmoid, bias=zero_t)
                        elif gate == "u":
                            u_sb[rt] = gate_pool.tile([P, 512], F32,
                                                      name="u_sb", tag="u")
                            nc.scalar.activation(out=u_sb[rt], in_=pcs[rt],
                                                 func=AF.Sigmoid, bias=neg1_t)
                        else:
                            rc = blend_pool.tile([P, 512], F32, name="rc", tag="rc")
                            nc.vector.tensor_mul(rc, r_sb[rt], pcs[rt])
                            c_sb[rt] = gate_pool.tile([P, 512], F32,
                                                      name="c_sb", tag="c")
                            nc.scalar.activation(out=c_sb[rt], in_=rc, func=AF.Tanh,
                                                 bias=zero_t)
                # blend: d_new = d + u*(c - d)
                for rt in range(NRT):
                    rs = slice(rt * P, (rt + 1) * P)
                    cs = slice(g * 512, (g + 1) * 512)
                    dre = dre_pool.tile([P, 512], F32, name="dre", tag="dre")
                    nc.sync.dma_start(out=dre, in_=d_f32[rs, cs])
                    t = blend_pool.tile([P, 512], F32, name="t_blend", tag="t")
                    nc.vector.tensor_sub(t, c_sb[rt], dre)
                    nc.vector.tensor_mul(t, u_sb[rt], t)
                    nc.vector.tensor_add(t, t, dre)
                    nc.sync.dma_start(out=out[rs, cs], in_=t)


def build_nc():
    nc = bacc.Bacc()
    io = {
        "s_bf": nc.declare_dram_parameter("s_bf", [R, STOCH], BF16, isOutput=False),
        "a_bf": nc.declare_dram_parameter("a_bf", [R, ACTD], BF16, isOutput=False),
        "d_bf": nc.declare_dram_parameter("d_bf", [R, DETER], BF16, isOutput=False),
        "d_f32": nc.declare_dram_parameter("d_f32", [R, DETER], F32, isOutput=False),
        "Ws": nc.declare_dram_parameter("Ws", [STOCH, HID], BF16, isOutput=False),
        "Wa": nc.declare_dram_parameter("Wa", [ACTD, HID], BF16, isOutput=False),
        "Wd": nc.declare_dram_parameter("Wd", [DETER, HID], BF16, isOutput=False),
        "Whid": nc.declare_dram_parameter("Whid", [G, BLK_IN, BH], BF16,
                                          isOutput=False),
        "Wgate": nc.declare_dram_parameter("Wgate", [G, BH, 3 * BH], BF16,
                                           isOutput=False),
        "out": nc.declare_dram_parameter("out", [R, DETER], F32, isOutput=True),
    }
    aps = {k: v[:] for k, v in io.items()}
    with tile.TileContext(nc) as tc:
        _emit(nc, tc, aps)
    nc.compile()
    return nc


_NC = None


def _get_nc():
    global _NC
    if _NC is None:
        _NC = build_nc()
    return _NC


def make_in_maps(inputs):
    s = np.ascontiguousarray(np.asarray(inputs["s"], np.float32).reshape(N_ROWS, STOCH))
    a = np.ascontiguousarray(np.asarray(inputs["a"], np.float32).reshape(N_ROWS, ACTD))
    d = np.ascontiguousarray(np.asarray(inputs["d"], np.float32).reshape(N_ROWS, DETER))

    # The device kernel folds out LN gains/biases and the gate bias, which are
    # constants (ones/zeros) in this problem.  Verify.
    for nm, want in [("ln_s_g", 1), ("ln_a_g", 1), ("ln_d_g", 1), ("ln_h_g", 1),
                     ("ln_s_b", 0), ("ln_a_b", 0), ("ln_d_b", 0), ("ln_h_b", 0),
                     ("b_gate", 0)]:
        v = np.asarray(inputs[nm], np.float32)
        if not np.all(v == want):
            raise ValueError(f"kernel assumes {nm} == {want}; got varying values")

    w_bf = {
        "Ws": np.asarray(inputs["W_s"], np.float32).astype(NP_BF16),
        "Wa": np.asarray(inputs["W_a"], np.float32).astype(NP_BF16),
        "Wd": np.asarray(inputs["W_d"], np.float32).astype(NP_BF16),
        "Whid": np.asarray(inputs["W_hid"], np.float32).astype(NP_BF16),
        "Wgate": np.asarray(inputs["W_gate"], np.float32).astype(NP_BF16),
    }
    in_maps = []
    for c in range(NCORES):
        rows = slice(c * R, (c + 1) * R)
        in_maps.append({
            "s_bf": s[rows].astype(NP_BF16),
            "a_bf": a[rows].astype(NP_BF16),
            "d_bf": d[rows].astype(NP_BF16),
            "d_f32": d[rows],
            **w_bf,
        })
    return in_maps


def run(inputs, **spmd_kwargs):
    nc = _get_nc()
    in_maps = make_in_maps(inputs)
    res = run_bass_kernel_spmd(nc, in_maps, core_ids=list(range(NCORES)),
                               **spmd_kwargs)
    outs = [np.asarray(res.results[c]["out"], np.float32) for c in range(NCORES)]
    full = np.concatenate(outs, axis=0).reshape(B, T, DETER)
    return full, res


def kernel(**inputs) -> np.ndarray:
    full, _ = run(inputs)
    return full
